# revision 1
# baseline (speedup 1.0000x reference)
"""Trainium2 Bass kernel for nn_CausalStructureLearner.

adjacency[b,i,j] = sigmoid(sum_h W2[h]*relu(ai[b,i,h]+aj[b,j,h]+b1[h]) + b2) * (1-eye)
structural = broadcast(structure_params)

Per core (batch sharded 4/core across 8 cores), fp16 hot path:
  Host folding: s_h = |W2[h]| goes into W1a/W1b/b1 so the h-reduction
  weight is sign(W2[h]) * I_128 (two constant +/-identity tiles; Ldweights
  are free and matmul cost is out-free-size only).  W_enc is folded into
  both W1 halves (wenca = W_enc@W1a', wencb = W_enc@W1b') so the nf
  staging disappears: aj and ai are single accumulation chains straight
  from the transposed input, with biases folded in via host precompute
  (ai's bias rides a ones-row matmul step).  All constants ride in three
  packed DMAs.
  prep (PE): cfb -> ajb [h,j] and ai [i,h]; ajb round-trips through DRAM
  so rows can be partition-broadcast.  Each chain's first quarter-octet
  broadcast issues right after its ajb write; quarters (4 rows) fill the
  pipeline fast, then full octets (8 rows) stream with a 2-3 cycle
  prefetch lead, emissions spread across the cycle.
  main: four per-batch PSUM accumulation chains over h, interleaved
  round-robin and skewed one step apart (chain b handles h = g-b):
    DMA:  broadcast ajb rows across 128 partitions (fp16)
    ACT (chain 0 t0) / Pool (chain 0 t1) / DVE (chains 1-3):
          hid[:,t,:] = relu(bcast + ai[:,t,h] per-partition bias)
          (DVE runs in 4x mode: 2-byte packed, all-SBUF)
    PE:   ps_adj[b] += sign(W2[h]) * I @ hid   ([128,512] fp32 accumulate)
  Chain 0 (the ACT/Pool chain) retires first so ACT's tail is just the
  four sigmoids.
  post (inlined as each chain ends): ACT sigmoid(+b2) from PSUM -> fp16
  SBUF -> DMA out.  The diagonal zero and the fp32 upcast happen on the
  host during unshard (like the structural broadcast).

_split_waits(): this container's neuronxcc walrus accepts only one
sync-wait per ISA instruction; extras are hoisted into standalone
EventSemaphore instructions on the same engine.
"""

import os
import sys

sys.path.insert(0, "/opt/trn_rl_repo")

import numpy as np

import bass_rust
import concourse.bass as bass
import concourse.tile as tile
from concourse import mybir
from concourse.bass_utils import run_bass_kernel_spmd

B, N, F_, H = 32, 256, 256, 64
NCORES = 8
BPC = B // NCORES  # batches per core
P = 128  # partitions
ACT_CHAIN = 0  # chain whose hid ops run on ACT (ends first)

_CACHE = {}
LAST_RESULT = None  # test harness can read exec_time_ns from here


def _bcast_rows(ap, nparts):
    """AP that reads a [k, n] slice broadcast to [nparts, k, n] partitions.

    Used as DMA source: out[p, k, n] = in[k, n] for all p.
    """
    return bass.AP(
        tensor=ap.tensor,
        offset=ap.offset,
        ap=[[0, nparts]] + [list(d) for d in ap.ap],
    )


def _split_waits(nc, keep=1):
    """Walrus (neuronxcc codegen) only supports one sync-wait per ISA
    instruction; Tile emits several. Hoist extras into standalone
    EventSemaphore instructions on the same engine, just before."""
    n = 0
    for f in nc.m.functions:
        for blk in f.blocks:
            new = []
            for ins in blk.instructions:
                si = ins.sync_info
                if si is not None and len(si.on_wait) > keep:
                    extra, kept = si.on_wait[:-keep], si.on_wait[-keep:]
                    for w in extra:
                        ev = mybir.InstEventSemaphore(name=f"I-wsplit-{n}")
                        n += 1
                        ev.engine = ins.engine
                        ev.sync_info = bass_rust.SyncInfo(on_wait=[w], on_update=[])
                        new.append(ev)
                    ins.sync_info = bass_rust.SyncInfo(
                        on_wait=kept, on_update=si.on_update
                    )
                new.append(ins)
            blk.instructions = new
    return n


def _build():
    nc = bass.Bass()
    f32 = mybir.dt.float32
    f16 = mybir.dt.float16
    bf16 = mybir.dt.float16  # fp16: same engine throughput as bf16, 8x mantissa

    # ---- DRAM tensors (per-core inputs) ----
    cfb = nc.dram_tensor("cfb", [BPC, F_, N], bf16, kind="ExternalInput")
    # packed fp16 consts: [:, 0:128] wencb=W_enc@W1b' (2 k-blocks of 64),
    # [:, 128:256] wenca=W_enc@W1a' (2 k-blocks), [64, 256:320] bias_a,
    # [64, 320:448] ones, [:, 256:384]+[:, 384:512] are +I / -I for rows!=64
    # (row 64 of the identities is rebuilt by never using column block 256:512
    # at row 64) -- identities moved to a second fp16 panel instead
    cpack16 = nc.dram_tensor("cpack16", [P, 512], bf16, kind="ExternalInput")
    cpack16b = nc.dram_tensor("cpack16b", [P, 256], bf16, kind="ExternalInput")
    # packed fp32 consts: col 0 benc (parts 0-63), col 1 b1' (parts 0-63),
    # col 2 b2 (all parts)
    cpack32 = nc.dram_tensor("cpack32", [P, 3], f32, kind="ExternalInput")
    adj = nc.dram_tensor("adj", [BPC, N, N], f16, kind="ExternalOutput")
    # internal DRAM scratch used to broadcast ajb rows across partitions
    ajb_d = nc.dram_tensor("ajb_d", [BPC, H, N], bf16)

    AF = mybir.ActivationFunctionType
    OP = mybir.AluOpType

    with tile.TileContext(nc) as tc:
        with (
            tc.tile_pool(name="consts", bufs=1) as consts,
            tc.tile_pool(name="prep", bufs=4) as prep,
            tc.tile_pool(name="small", bufs=4) as small,
            tc.tile_pool(name="in0p", bufs=16) as in0p,
            tc.tile_pool(name="in0qp", bufs=8) as in0qp,
            tc.tile_pool(name="hidp", bufs=8) as hidp,
            tc.tile_pool(name="hidap", bufs=4) as hidap,
            tc.tile_pool(name="outp", bufs=8) as outp,
            tc.tile_pool(name="pprep", bufs=3, space="PSUM") as pprep,
            tc.tile_pool(name="padj", bufs=1, space="PSUM") as padj,
        ):
            # ---- first input + const loads (SP queue, no compute waits);
            # cfb for later batches loads inside the prep loop to spread the
            # head DMA burst ----
            cfbT_all = {}
            c16 = consts.tile([P, 512], bf16)
            nc.sync.dma_start(out=c16, in_=cpack16[:])
            cfbT = prep.tile([P, 2, N], bf16, tag="cfbT")
            nc.sync.dma_start(out=cfbT, in_=cfb[0].rearrange("(k p) i -> p k i", p=P))
            cfbT_all[0] = cfbT
            c32 = consts.tile([P, 3], f32)
            nc.sync.dma_start(out=c32, in_=cpack32[:])
            c16b = consts.tile([P, 256], bf16)
            nc.sync.dma_start(out=c16b, in_=cpack16b[:])

            wencb_sb = c16[:, 0:128].rearrange("p (k h) -> p k h", k=2)
            wenca_sb = c16[:, 128:256].rearrange("p (k h) -> p k h", k=2)
            biasa_sb = c16[64:65, 256:320]
            ones_sb = c16[64:65, 320:448]
            wsig_sb = c16b[:, 0:256].rearrange("p (k q) -> p k q", k=2)
            b1_sb = c32[0:H, 1:2]
            b2_sb = c32[:, 2:3]

            HB = 8  # h-rows per broadcast octet
            NOCT = H // HB
            # hid t-op engine map: chain 0 runs t0 on ACT and t1 on Pool
            # (similar per-op speed, parallel engines); chains 1-3 on DVE.
            def hid_engine(b, t, h):
                if b == ACT_CHAIN:
                    return "a" if t == 0 else "p"
                return "d"

            in0s = {}

            def emit_bcast(b, o):
                in0 = in0p.tile([P, HB, N], bf16, tag="in0")
                nc.sync.dma_start(
                    out=in0, in_=_bcast_rows(ajb_d[b, o * HB : (o + 1) * HB, :], P)
                )
                in0s[(b, o)] = in0

            def emit_bcast_q(b, q):
                # quarter-size first transfers: all chains get rows 0-3
                # quickly, then rows 4-7, so the pipeline fill keeps pace
                in0 = in0qp.tile([P, HB // 2, N], bf16, tag="in0q")
                nc.sync.dma_start(
                    out=in0,
                    in_=_bcast_rows(ajb_d[b, q * 4 : (q + 1) * 4, :], P),
                )
                in0s[(b, "q%d" % q)] = in0

            prep_out = [None] * BPC

            def emit_ai(b):
                # ---- ai [i, h] = cfb @ (W_enc@W1a')  (+ bias_a via ones row)
                use_act = b == ACT_CHAIN
                cfbT = cfbT_all[b]
                ai_t = small.tile([P, 2, H], f32, tag="ai_a" if use_act else "ai_d")
                for t in range(2):
                    ps_ai = pprep.tile([P, H], f32, tag="pp")
                    for k in range(2):
                        nc.tensor.matmul(
                            ps_ai,
                            cfbT[:, k, t * P : (t + 1) * P],
                            wenca_sb[:, k, :],
                            start=(k == 0),
                            stop=False,
                        )
                    nc.tensor.matmul(
                        ps_ai,
                        ones_sb,
                        biasa_sb,
                        start=False,
                        stop=True,
                    )
                    if use_act:
                        nc.scalar.copy(ai_t[:, t, :], ps_ai)
                    else:
                        nc.vector.tensor_copy(ai_t[:, t, :], ps_ai)
                prep_out[b] = ai_t

            for b in range(BPC):
                use_act = b == ACT_CHAIN
                if b + 1 < BPC:
                    nxt = prep.tile([P, 2, N], bf16, tag="cfbT")
                    nc.sync.dma_start(
                        out=nxt, in_=cfb[b + 1].rearrange("(k p) i -> p k i", p=P)
                    )
                    cfbT_all[b + 1] = nxt
                # ---- ajT [h, j] = (W_enc@W1b').T @ cfb.T  (+ bias_b) ----
                cfbT = cfbT_all[b]
                ps_aj = pprep.tile([H, N], f32, tag="pp")
                for k in range(2):
                    nc.tensor.matmul(
                        ps_aj,
                        wencb_sb[:, k, :],
                        cfbT[:, k, :],
                        start=(k == 0),
                        stop=(k == 1),
                    )
                ajb_sb = small.tile([H, N], bf16, tag="ajb")
                nc.vector.tensor_scalar(ajb_sb, ps_aj, b1_sb, None, OP.add)
                nc.sync.dma_start(out=ajb_d[b], in_=ajb_sb)

                # first octet broadcasts for this chain issue immediately so
                # the main loop's head isn't serialized behind later preps
                emit_bcast_q(b, 0)
                emit_ai(b)

            for b in range(BPC):
                emit_bcast_q(b, 1)
            for b in range(BPC):
                emit_bcast(b, 1)
            for b in range(BPC):
                emit_bcast(b, 2)

            # ---- main: 4 interleaved accumulation chains, h-outer ----
            ps_adj_all = []
            for bb in range(BPC):
                ps_adj = padj.tile([P, 2 * N], f32, tag=f"ps_adj{bb}")
                ps_adj_all.append(ps_adj)
            # skewed steps: chain b processes h = g - b, so chain ends
            # stagger and post-processing overlaps the remaining chains
            SKEW = 1
            for g in range(H + SKEW * (BPC - 1)):
                # prefetch octet o for chain b two cycles ahead; emissions
                # spread across the cycle so transfers don't cluster
                for b in range(BPC):
                    o = g // HB + 3
                    if g % HB == (2 * b + 1) % HB and o < NOCT and (b, o) not in in0s:
                        emit_bcast(b, o)

                for b in range(BPC):
                    h = g - SKEW * b
                    if not (0 <= h < H):
                        continue
                    if h < HB:
                        key = "q%d" % (h // 4)
                        coff = h % 4
                    else:
                        key = h // HB
                        coff = h % HB
                    ai_t = prep_out[b]
                    if b == ACT_CHAIN:
                        hid = hidap.tile([P, 2, N], bf16, tag="hid_a")
                    else:
                        hid = hidp.tile([P, 2, N], bf16, tag="hid")
                    in0 = in0s[(b, key)]
                    for t in range(2):
                        eng = hid_engine(b, t, h)
                        if eng == "a":
                            nc.scalar.activation(
                                hid[:, t, :], in0[:, coff, :], AF.Relu,
                                bias=ai_t[:, t, h : h + 1], scale=1.0,
                            )
                        elif eng == "p":
                            nc.gpsimd.tensor_scalar(
                                hid[:, t, :], in0[:, coff, :],
                                ai_t[:, t, h : h + 1], 0.0,
                                OP.add, OP.max,
                            )
                        else:
                            nc.vector.tensor_scalar(
                                hid[:, t, :], in0[:, coff, :],
                                ai_t[:, t, h : h + 1], 0.0,
                                OP.add, OP.max,
                            )
                    nc.tensor.matmul(
                        ps_adj_all[b],
                        wsig_sb[:, _SIGN_SEL[h], :],
                        hid,
                        start=(h == 0),
                        stop=(h == H - 1),
                    )

                if g >= H - 1 and (g - (H - 1)) % SKEW == 0:
                    b = (g - (H - 1)) // SKEW
                    sig = outp.tile([P, 2, N], f16, tag="sig")
                    nc.scalar.activation(
                        sig, ps_adj_all[b], AF.Sigmoid, bias=b2_sb, scale=1.0
                    )
                    nc.sync.dma_start(
                        out=adj[b].rearrange("(t p) j -> p t j", p=P), in_=sig
                    )

    _split_waits(nc)
    return nc


# sign selection per h is baked into the instruction stream; it is fixed
# before _build() runs from the actual W2 input.
_SIGN_SEL = [0] * H


def kernel(causal_factors_batch, W_enc, b_enc, W1, b1, W2, b2, structure_params):
    global LAST_RESULT, _SIGN_SEL
    cfb = np.asarray(causal_factors_batch, dtype=np.float32)
    W_enc = np.asarray(W_enc, dtype=np.float32)
    b_enc = np.asarray(b_enc, dtype=np.float32)
    W1 = np.asarray(W1, dtype=np.float32)
    b1 = np.asarray(b1, dtype=np.float32)
    W2 = np.asarray(W2, dtype=np.float32)
    b2 = np.asarray(b2, dtype=np.float32)
    structure_params = np.asarray(structure_params, dtype=np.float32)

    bf = np.float16
    w2f = W2.reshape(-1)
    s_h = np.abs(w2f)  # folded into W1a/W1b/b1; sign goes into the weights
    _SIGN_SEL = [int(x) for x in (w2f < 0)]

    key = ("nc", tuple(_SIGN_SEL))
    if key not in _CACHE:
        _CACHE[key] = _build()
    nc = _CACHE[key]
    _CACHE["nc"] = nc  # canonical handle for the test harness

    w1a_s = W1[:H] * s_h[None, :]
    w1b_s = W1[H:] * s_h[None, :]
    wenca = W_enc @ w1a_s  # [F, H]
    wencb = W_enc @ w1b_s
    bias_a = b_enc @ w1a_s  # [H]
    bias_b = b_enc @ w1b_s + b1 * s_h
    cp16 = np.zeros((P, 512), dtype=bf)
    cp16[:, 0:128] = wencb.reshape(2, P, H).transpose(1, 0, 2).reshape(P, 128)
    cp16[:, 128:256] = wenca.reshape(2, P, H).transpose(1, 0, 2).reshape(P, 128)
    cp16[64, 256:320] = bias_a.astype(bf)
    cp16[64, 320:448] = 1.0
    eye = np.eye(P, dtype=np.float32)
    cp16b = np.concatenate([eye, -eye], axis=1).astype(bf)
    cp32 = np.zeros((P, 3), dtype=np.float32)
    cp32[0:H, 1] = bias_b
    cp32[:, 2] = float(b2.reshape(-1)[0])
    shared = {"cpack16": cp16, "cpack16b": cp16b, "cpack32": cp32}
    in_maps = []
    for c in range(NCORES):
        m = dict(shared)
        m["cfb"] = np.ascontiguousarray(
            cfb[c * BPC : (c + 1) * BPC].transpose(0, 2, 1)
        ).astype(np.float16)
        in_maps.append(m)

    trace = bool(os.environ.get("BASS_TRACE"))
    res = run_bass_kernel_spmd(nc, in_maps, list(range(NCORES)), trace=trace)
    LAST_RESULT = res

    adjacency = np.concatenate(
        [res.results[c]["adj"].astype(np.float32) for c in range(NCORES)], axis=0
    )
    adjacency[:, np.arange(N), np.arange(N)] = 0.0
    structural = np.broadcast_to(structure_params, (B, N, N)).astype(np.float32).copy()
    return adjacency, structural

